# revision 11
# baseline (speedup 1.0000x reference)
"""DogeCDMoE Trainium2 kernel: product-key MoE routing + dense MLP.

Strategy (8 NeuronCores, data-parallel over the 4096 tokens, 512 each):
  - Host: compose `keys` into W_q so routing scores come from one PE matmul;
    pre-transpose weights; fp8(e4m3) quantize the expert path (hidden states,
    composed routing keys, down_embed, up_embed) with power-of-2 scaling so
    the all-expert logit and expert-combine matmuls run in DoubleRow fp8
    mode (2 MACs/cell/cycle); dense up/down stay bf16 for accuracy.
  - Device per core (512 tokens = 4 chunks of 128):
      phase B (per chunk): sim + all-expert logits via DoubleRow fp8 matmuls
        sharing the same stationary hidden-state tile; top-8 per head/axis on
        DVE (max8/max_index), cartesian top-8 via the product-key bound,
        expert ids rebuilt with int ALU ops, softmax on the 8 scores.
      phase A5 (per chunk): probs of all 4 heads scattered in ONE GPSIMD
        local_scatter per expert-quarter (cross-head duplicate experts
        resolve last-wins; analytically negligible), dense mult by logits,
        DMA-xbar transpose, then ACT silu directly into the fp8 S.T tile.
      phase C: dense up-proj (bf16) + silu -> y1T.
      phase D: down-proj (bf16) and expert combine (DoubleRow fp8)
        accumulate into the same 8 PSUM banks, output in natural [tok, d]
        orientation.
"""

import numpy as np
import ml_dtypes

B, T, H = 2, 2048, 1024
I = 4096
HEADS = 4
RET = 128
E = 4096           # NUM_EXPERTS
NK = 64            # NUM_KEYS
K = 8
NCORES = 8
NT = (B * T) // NCORES   # 512 tokens per core
P = 128
TCH = NT // P            # 4 token chunks
HK = H // P              # 8 contraction chunks over H
KK = H // 256            # 4 DoubleRow contraction chunks over H
ICH = I // P             # 32 chunks over intermediate / expert dim
EC2 = E // 256           # 16 DoubleRow chunks over expert dim
QE = 1024                # local_scatter quarter size over expert dim
FPS = 64.0               # power-of-2 fp8 pre-scale for wk / de

_CACHE = {}


def _build_program(repeat=1):
    from contextlib import ExitStack
    import concourse.tile as tile
    from concourse import bacc, mybir

    nc = bacc.Bacc("TRN2", target_bir_lowering=False, debug=False)
    f32 = mybir.dt.float32
    bf16 = mybir.dt.bfloat16
    f8 = mybir.dt.float8e4
    i32 = mybir.dt.int32
    i16 = mybir.dt.int16
    u32 = mybir.dt.uint32
    AF = mybir.ActivationFunctionType
    OP = mybir.AluOpType
    AX = mybir.AxisListType
    DR = mybir.MatmulPerfMode.DoubleRow

    # ---- I/O ----
    hs8_d = nc.dram_tensor("hs8", [P, KK, 2, NT], f8, kind="ExternalInput")
    hsT_b = nc.dram_tensor("hsT_b", [H, NT], bf16, kind="ExternalInput")
    wk_d = nc.dram_tensor("wk8", [P, KK, 2, 512], f8, kind="ExternalInput")
    de_d = nc.dram_tensor("de8", [P, KK, 2, E], f8, kind="ExternalInput")
    wupT_d = nc.dram_tensor("wupT", [H, I], bf16, kind="ExternalInput")
    wdownT_d = nc.dram_tensor("wdownT", [I, H], bf16, kind="ExternalInput")
    ue_d = nc.dram_tensor("ue8", [P, EC2, 2, H], f8, kind="ExternalInput")
    out_d = nc.dram_tensor("out", [NT, H], f32, kind="ExternalOutput")

    wupT_r = wupT_d[:].rearrange("(o p) i -> p o i", p=P)
    wdownT_r = wdownT_d[:].rearrange("(o p) d -> p o d", p=P)

    with tile.TileContext(nc) as tc, ExitStack() as ctx:
        res = ctx.enter_context(tc.tile_pool(name="res", bufs=1))
        streams = ctx.enter_context(tc.tile_pool(name="streams", bufs=2))
        wstream = ctx.enter_context(tc.tile_pool(name="wstream", bufs=3))
        lgpool = ctx.enter_context(tc.tile_pool(name="lgpool", bufs=3))
        rpool = ctx.enter_context(tc.tile_pool(name="rpool", bufs=2))
        scpool = ctx.enter_context(tc.tile_pool(name="scpool", bufs=2))
        stpool = ctx.enter_context(tc.tile_pool(name="stpool", bufs=2))
        outp = ctx.enter_context(tc.tile_pool(name="outp", bufs=2))
        psum = ctx.enter_context(tc.tile_pool(name="psum", bufs=8, space="PSUM"))

        # ---------- residents ----------
        iota8 = res.tile([P, 8], i32)
        nc.gpsimd.iota(iota8[:], pattern=[[1, 8]], base=0, channel_multiplier=0)

        wk_sb = res.tile([P, KK, 2, 512], f8)
        nc.scalar.dma_start(wk_sb[:], wk_d[:])
        # expert down-embeddings resident in fp8, split per kk for early start
        de_sb = res.tile([P, KK, 2, E], f8)
        for kk in range(KK):
            nc.sync.dma_start(de_sb[:, kk, :, :], de_d[:, kk, :, :])

        hs8_sb = res.tile([P, KK, 2, NT], f8)
        hsTb_sb = res.tile([P, HK, NT], bf16)

        y1T = res.tile([P, ICH, NT], bf16)         # silu(up-proj), I on partitions
        sT8 = res.tile([P, EC2, 2, NT], f8)        # S.T in DR-fp8 pair layout

        p8bf = res.tile([P, TCH, HEADS, 8], bf16)  # softmax probs (all heads)
        idx16 = res.tile([P, TCH, 4, HEADS, 8], i16)  # per-quarter scatter idx

        hsTb_r = hsT_b[:].rearrange("(o p) n -> p o n", p=P)
        for _rep in range(repeat):
            for kk in range(KK):
                nc.sync.dma_start(hs8_sb[:, kk, :, :], hs8_d[:, kk, :, :])
            nc.sync.dma_start(hsTb_sb[:, :HK // 2, :], hsTb_r[:, :HK // 2, :])
            nc.sync.dma_start(hsTb_sb[:, HK // 2:, :], hsTb_r[:, HK // 2:, :])

            # ---------- phase B: sim + all-expert logits, DoubleRow fp8.
            # kk-outer passes reuse the stationary hidden-state tile across
            # 3-4 matmuls per LDWEIGHTS; pass widths (4,3,3) keep at most 7
            # PSUM banks in flight (9 live banks on the 8-buf pool corrupts).
            for c in range(TCH):
                lg = lgpool.tile([P, E], bf16, tag="lg", name=f"lg{c}")
                hs_c = hs8_sb[:, :, :, c * P:(c + 1) * P]
                sim = rpool.tile([P, 512], f32, tag="sim")
                for ecs in ((-1, 0, 1, 2), (3, 4, 5), (6, 7)):
                    pss = [psum.tile([P, 512], f32, tag="ps", name=f"ps{c}_{ec}")
                           for ec in ecs]
                    for kk in range(KK):
                        for ps, ec in zip(pss, ecs):
                            mov = (wk_sb[:, kk, :, :] if ec < 0 else
                                   de_sb[:, kk, :, ec * 512:(ec + 1) * 512])
                            nc.tensor.matmul(ps[:], hs_c[:, kk, :, :], mov,
                                             start=(kk == 0), stop=(kk == KK - 1),
                                             perf_mode=DR)
                    for ps, ec in zip(pss, ecs):
                        if ec < 0:
                            # scores stay 64x-scaled; Exp descales later
                            nc.vector.tensor_copy(sim[:], ps[:])
                        elif ec % 2 == 0:
                            nc.scalar.activation(lg[:, ec * 512:(ec + 1) * 512],
                                                 ps[:], AF.Copy, scale=1.0 / FPS)
                        else:
                            # odd ec drains on DVE so bank recycling isn't
                            # serialized behind a single engine
                            nc.vector.tensor_scalar(
                                lg[:, ec * 512:(ec + 1) * 512], ps[:],
                                1.0 / FPS, None, op0=OP.mult)

                # ---- routing on the (64x-scaled) sim scores ----
                e8i = rpool.tile([P, HEADS, 8], i32, tag="e8i")
                for h in range(HEADS):
                    simx = sim[:, h * NK:(h + 1) * NK]
                    simy = sim[:, 256 + h * NK:256 + (h + 1) * NK]
                    sx = rpool.tile([P, 8], f32, tag="sx")
                    sy = rpool.tile([P, 8], f32, tag="sy")
                    ix = rpool.tile([P, 8], u32, tag="ix")
                    iy = rpool.tile([P, 8], u32, tag="iy")
                    nc.vector.max(sx[:], simx)
                    nc.vector.max_index(ix[:], sx[:], simx)
                    nc.vector.max(sy[:], simy)
                    nc.vector.max_index(iy[:], sy[:], simy)

                    cc = rpool.tile([P, 8, 8], f32, tag="cc")
                    nc.vector.tensor_tensor(cc[:], sx[:, :, None].to_broadcast([P, 8, 8]),
                                            sy[:, None, :].to_broadcast([P, 8, 8]), OP.add)
                    cflat = cc[:].rearrange("p a b -> p (a b)")
                    s8 = rpool.tile([P, 8], f32, tag="s8")
                    pk = rpool.tile([P, 8], u32, tag="pk")
                    nc.vector.max(s8[:], cflat)
                    nc.vector.max_index(pk[:], s8[:], cflat)

                    # softmax over the 8 selected scores (descale inside Exp)
                    d8 = rpool.tile([P, 8], f32, tag="d8")
                    nc.vector.tensor_scalar(d8[:], s8[:], s8[:, 0:1], None, op0=OP.subtract)
                    ex8 = rpool.tile([P, 8], f32, tag="ex8")
                    nc.scalar.activation(ex8[:], d8[:], AF.Exp, scale=1.0 / FPS)
                    z = rpool.tile([P, 1], f32, tag="z")
                    nc.vector.tensor_reduce(z[:], ex8[:], axis=AX.X, op=OP.add)
                    rz = rpool.tile([P, 1], f32, tag="rz")
                    nc.vector.reciprocal(rz[:], z[:])
                    nc.vector.tensor_scalar(p8bf[:, c, h, :], ex8[:], rz[:, 0:1], None,
                                            op0=OP.mult)

                    # expert ids: e8 = ix[pk>>3]*64 + iy[pk&7]
                    pkhu = rpool.tile([P, 8], u32, tag="pkhu")
                    pklu = rpool.tile([P, 8], u32, tag="pklu")
                    nc.vector.tensor_scalar(pkhu[:], pk[:], 3, None, op0=OP.logical_shift_right)
                    nc.vector.tensor_scalar(pklu[:], pk[:], 7, None, op0=OP.bitwise_and)
                    pkh = rpool.tile([P, 8], i32, tag="pkh")
                    pkl = rpool.tile([P, 8], i32, tag="pkl")
                    nc.vector.tensor_copy(pkh[:], pkhu[:])
                    nc.vector.tensor_copy(pkl[:], pklu[:])
                    ixi = rpool.tile([P, 8], i32, tag="ixi")
                    iyi = rpool.tile([P, 8], i32, tag="iyi")
                    nc.vector.tensor_copy(ixi[:], ix[:])
                    nc.vector.tensor_copy(iyi[:], iy[:])

                    ohx = rpool.tile([P, 8, 8], i32, tag="ohx")
                    ohy = rpool.tile([P, 8, 8], i32, tag="ohy")
                    nc.vector.tensor_tensor(ohx[:], pkh[:, :, None].to_broadcast([P, 8, 8]),
                                            iota8[:, None, :].to_broadcast([P, 8, 8]), OP.is_equal)
                    nc.vector.tensor_tensor(ohy[:], pkl[:, :, None].to_broadcast([P, 8, 8]),
                                            iota8[:, None, :].to_broadcast([P, 8, 8]), OP.is_equal)
                    mx = rpool.tile([P, 8, 8], i32, tag="mx")
                    my = rpool.tile([P, 8, 8], i32, tag="my")
                    nc.vector.tensor_tensor(mx[:], ohx[:],
                                            ixi[:, None, :].to_broadcast([P, 8, 8]), OP.mult)
                    nc.vector.tensor_tensor(my[:], ohy[:],
                                            iyi[:, None, :].to_broadcast([P, 8, 8]), OP.mult)
                    ixs = rpool.tile([P, 8], i32, tag="ixs")
                    iys = rpool.tile([P, 8], i32, tag="iys")
                    with nc.allow_low_precision(reason="int32 onehot-select, exact"):
                        nc.vector.tensor_reduce(ixs[:], mx[:], axis=AX.X, op=OP.add)
                        nc.vector.tensor_reduce(iys[:], my[:], axis=AX.X, op=OP.add)
                    nc.vector.scalar_tensor_tensor(e8i[:, h, :], ixs[:], NK, iys[:],
                                                   op0=OP.mult, op1=OP.add)

                # quarter-local scatter indices, wrong-quarter -> negative int16
                for q in range(4):
                    t1 = rpool.tile([P, HEADS, 8], i32, tag="t1")
                    nc.vector.tensor_scalar(t1[:], e8i[:], QE * q + QE, None,
                                            op0=OP.subtract)
                    t2 = rpool.tile([P, HEADS, 8], i32, tag="t2")
                    nc.vector.tensor_scalar(t2[:], t1[:], 4095, None,
                                            op0=OP.bitwise_and)
                    nc.vector.tensor_scalar(idx16[:, c, q, :, :], t2[:], 3072, None,
                                            op0=OP.subtract)

                # ---- phase A5: scatter all 4 heads at once, mult, transpose,
                #      silu straight into the fp8 S.T layout ----
                pq = scpool.tile([P, E], bf16, tag="pq")
                for q in range(4):
                    nc.gpsimd.local_scatter(
                        pq[:, q * QE:(q + 1) * QE], p8bf[:, c, :, :],
                        idx16[:, c, q, :, :], channels=P, num_elems=QE,
                        num_idxs=HEADS * 8)
                tt = scpool.tile([P, E], bf16, tag="tt")
                nc.vector.tensor_tensor(tt[:], lg[:], pq[:], OP.mult)
                stage = stpool.tile([P, ICH, P], bf16, tag="stage")
                # alternate dispatch queues so descriptor generation for the
                # transpose doesn't serialize behind one sequencer
                eng = nc.scalar if c % 2 == 0 else nc.sync
                eng.dma_start_transpose(stage[:], tt[:])
                nc.scalar.activation(
                    sT8[:, :, :, c * P:(c + 1) * P],
                    stage[:].rearrange("p (a b) m -> p a b m", b=2), AF.Silu)

            # ---------- phase C: dense up-proj + silu ----------
            for ic4 in range(ICH // 2):
                wup_t = streams.tile([P, HK, 2 * P], bf16, tag="wup_t")
                nc.sync.dma_start(wup_t[:], wupT_r[:, :, ic4 * 2 * P:(ic4 + 1) * 2 * P])
                for j in range(2):
                    ic = ic4 * 2 + j
                    ps = psum.tile([P, 512], f32, tag="ps")
                    for kk in range(HK):
                        nc.tensor.matmul(ps[:], wup_t[:, kk, j * P:(j + 1) * P],
                                         hsTb_sb[:, kk, :],
                                         start=(kk == 0), stop=(kk == HK - 1))
                    nc.scalar.activation(y1T[:, ic, :], ps[:], AF.Silu)

            # ---------- phase D: down-proj (bf16) + expert combine (DR fp8),
            #            fused in the same PSUM banks, [tok, d] out ----------
            ps_d = [psum.tile([P, 512], f32, tag="ps", name=f"ps_d{c}_{dh}")
                    for c in range(TCH) for dh in range(2)]
            for ic in range(ICH):
                wd_t = wstream.tile([P, H], bf16, tag="wd_t")
                nc.sync.dma_start(wd_t[:], wdownT_r[:, ic, :])
                for c in range(TCH):
                    for dh in range(2):
                        nc.tensor.matmul(ps_d[c * 2 + dh][:],
                                         y1T[:, ic, c * P:(c + 1) * P],
                                         wd_t[:, dh * 512:(dh + 1) * 512],
                                         start=(ic == 0), stop=False)
            for ic2 in range(EC2):
                ue_t = wstream.tile([P, 2, H], f8, tag="ue_t")
                # ACT's DMA queue is idle during phase D; keep wd on SP so
                # the two weight streams generate descriptors in parallel
                nc.scalar.dma_start(ue_t[:], ue_d[:, ic2, :, :])
                for c in range(TCH):
                    for dh in range(2):
                        nc.tensor.matmul(ps_d[c * 2 + dh][:],
                                         sT8[:, ic2, :, c * P:(c + 1) * P],
                                         ue_t[:, :, dh * 512:(dh + 1) * 512],
                                         start=False, stop=(ic2 == EC2 - 1),
                                         perf_mode=DR)
            for c in range(TCH):
                for dh in range(2):
                    ot = outp.tile([P, 512], f32, tag="ot")
                    if dh == 0:
                        nc.vector.tensor_copy(ot[:], ps_d[c * 2 + dh][:])
                    else:
                        nc.scalar.activation(ot[:], ps_d[c * 2 + dh][:], AF.Copy)
                    oeng = nc.gpsimd if dh == 0 else nc.sync
                    oeng.dma_start(
                        out_d[c * P:(c + 1) * P, dh * 512:(dh + 1) * 512], ot[:])

    nc.compile()
    return nc


def _host_prep(hidden_states, W_up, W_down, W_q, keys, down_embed, up_embed):
    bf = ml_dtypes.bfloat16
    f8 = ml_dtypes.float8_e4m3
    hs = np.asarray(hidden_states, dtype=np.float32).reshape(B * T, H)
    W_up = np.asarray(W_up, dtype=np.float32)
    W_down = np.asarray(W_down, dtype=np.float32)
    W_q = np.asarray(W_q, dtype=np.float32)
    keys = np.asarray(keys, dtype=np.float32)
    down_embed = np.asarray(down_embed, dtype=np.float32)
    up_embed = np.asarray(up_embed, dtype=np.float32)

    # compose product-key similarity: WK[(p2,h,k), d] = sum_r Wq[(p2,h,r), d]*keys[h,k,p2,r]
    Wq3 = W_q.reshape(2, HEADS, NK, H).astype(np.float64)
    WK = np.einsum("phrd,hkpr->phkd", Wq3, keys.astype(np.float64))
    WK_T = np.ascontiguousarray(WK.reshape(512, H).T.astype(np.float32))  # [H, 512]

    def dr_pack(m):  # [H, N] -> [128, KK, 2, N] with h = kk*256 + i*128 + p
        return np.ascontiguousarray(
            m.reshape(KK, 2, P, m.shape[1]).transpose(2, 0, 1, 3))

    shared = {
        "wk8": dr_pack(WK_T * FPS).astype(f8),
        "de8": dr_pack(np.ascontiguousarray(down_embed.T) * FPS).astype(f8),
        "wupT": np.ascontiguousarray(W_up.T).astype(bf),            # [H, I]
        "wdownT": np.ascontiguousarray(W_down.T).astype(bf),        # [I, H]
        # [128, EC2, 2, H] with e = ic2*256 + i*128 + p, raw scale
        "ue8": np.ascontiguousarray(
            up_embed.reshape(EC2, 2, P, H).transpose(2, 0, 1, 3)).astype(f8),
    }
    in_maps = []
    for i in range(NCORES):
        shard = hs[i * NT:(i + 1) * NT]                              # [NT, H]
        hsT = np.ascontiguousarray(shard.T)                          # [H, NT]
        m = dict(shared)
        m["hs8"] = dr_pack(hsT).astype(f8)
        m["hsT_b"] = hsT.astype(bf)
        in_maps.append(m)
    return in_maps


def kernel(hidden_states, W_up, W_down, W_q, keys, down_embed, up_embed,
           trace=False):
    from concourse.bass_utils import run_bass_kernel_spmd

    if "nc" not in _CACHE:
        _CACHE["nc"] = _build_program()
    nc = _CACHE["nc"]

    in_maps = _host_prep(hidden_states, W_up, W_down, W_q, keys,
                         down_embed, up_embed)
    res = run_bass_kernel_spmd(nc, in_maps, list(range(NCORES)), trace=trace)
    out = np.empty((B * T, H), np.float32)
    for i, r in enumerate(res.results):
        out[i * NT:(i + 1) * NT] = r["out"]
    if trace:
        kernel.last_results = res
    return out.reshape(B, T, H)


# revision 12
# speedup vs baseline: 1.0638x; 1.0638x over previous
"""DogeCDMoE Trainium2 kernel: product-key MoE routing + dense MLP.

Strategy (8 NeuronCores, data-parallel over the 4096 tokens, 512 each):
  - Host: compose `keys` into W_q so routing scores come from one PE matmul;
    pre-transpose weights; fp8(e4m3) quantize the expert path (hidden states,
    composed routing keys, down_embed, up_embed) with power-of-2 scaling so
    the all-expert logit and expert-combine matmuls run in DoubleRow fp8
    mode (2 MACs/cell/cycle); dense up/down stay bf16 for accuracy.
  - Device per core (512 tokens = 4 chunks of 128):
      phase B (per chunk): sim + all-expert logits via DoubleRow fp8 matmuls
        sharing the same stationary hidden-state tile; top-8 per head/axis on
        DVE (max8/max_index), cartesian top-8 via the product-key bound,
        expert ids rebuilt with int ALU ops, softmax on the 8 scores.
      phase A5 (per chunk): probs of all 4 heads scattered in ONE GPSIMD
        local_scatter per expert-quarter (cross-head duplicate experts
        resolve last-wins; analytically negligible), dense mult by logits,
        DMA-xbar transpose, then ACT silu directly into the fp8 S.T tile.
      phase C: dense up-proj (bf16) + silu -> y1T.
      phase D: down-proj (bf16) and expert combine (DoubleRow fp8)
        accumulate into the same 8 PSUM banks, output in natural [tok, d]
        orientation.
"""

import numpy as np
import ml_dtypes

B, T, H = 2, 2048, 1024
I = 4096
HEADS = 4
RET = 128
E = 4096           # NUM_EXPERTS
NK = 64            # NUM_KEYS
K = 8
NCORES = 8
NT = (B * T) // NCORES   # 512 tokens per core
P = 128
TCH = NT // P            # 4 token chunks
HK = H // P              # 8 contraction chunks over H
KK = H // 256            # 4 DoubleRow contraction chunks over H
ICH = I // P             # 32 chunks over intermediate / expert dim
EC2 = E // 256           # 16 DoubleRow chunks over expert dim
QE = 1024                # local_scatter quarter size over expert dim
FPS = 64.0               # power-of-2 fp8 pre-scale for wk / de

_CACHE = {}


def _build_program(repeat=1):
    from contextlib import ExitStack
    import concourse.tile as tile
    from concourse import bacc, mybir

    nc = bacc.Bacc("TRN2", target_bir_lowering=False, debug=False)
    f32 = mybir.dt.float32
    bf16 = mybir.dt.bfloat16
    f8 = mybir.dt.float8e4
    i32 = mybir.dt.int32
    i16 = mybir.dt.int16
    u32 = mybir.dt.uint32
    AF = mybir.ActivationFunctionType
    OP = mybir.AluOpType
    AX = mybir.AxisListType
    DR = mybir.MatmulPerfMode.DoubleRow

    # ---- I/O ----
    hs8_d = nc.dram_tensor("hs8", [P, KK, 2, NT], f8, kind="ExternalInput")
    hsT_b = nc.dram_tensor("hsT_b", [H, NT], bf16, kind="ExternalInput")
    wk_d = nc.dram_tensor("wk8", [P, KK, 2, 512], f8, kind="ExternalInput")
    de_d = nc.dram_tensor("de8", [P, KK, 2, E], f8, kind="ExternalInput")
    wupT_d = nc.dram_tensor("wupT", [H, I], bf16, kind="ExternalInput")
    wdownT_d = nc.dram_tensor("wdownT", [I, H], bf16, kind="ExternalInput")
    ue_d = nc.dram_tensor("ue8", [P, EC2, 2, H], f8, kind="ExternalInput")
    out_d = nc.dram_tensor("out", [NT, H], f32, kind="ExternalOutput")

    wupT_r = wupT_d[:].rearrange("(o p) i -> p o i", p=P)
    wdownT_r = wdownT_d[:].rearrange("(o p) d -> p o d", p=P)

    with tile.TileContext(nc) as tc, ExitStack() as ctx:
        res = ctx.enter_context(tc.tile_pool(name="res", bufs=1))
        streams = ctx.enter_context(tc.tile_pool(name="streams", bufs=2))
        wstream = ctx.enter_context(tc.tile_pool(name="wstream", bufs=3))
        lgpool = ctx.enter_context(tc.tile_pool(name="lgpool", bufs=3))
        rpool = ctx.enter_context(tc.tile_pool(name="rpool", bufs=2))
        scpool = ctx.enter_context(tc.tile_pool(name="scpool", bufs=2))
        stpool = ctx.enter_context(tc.tile_pool(name="stpool", bufs=2))
        outp = ctx.enter_context(tc.tile_pool(name="outp", bufs=2))
        psum = ctx.enter_context(tc.tile_pool(name="psum", bufs=8, space="PSUM"))

        # ---------- residents ----------
        iota8 = res.tile([P, 8], i32)
        nc.gpsimd.iota(iota8[:], pattern=[[1, 8]], base=0, channel_multiplier=0)

        wk_sb = res.tile([P, KK, 2, 512], f8)
        nc.scalar.dma_start(wk_sb[:], wk_d[:])
        # expert down-embeddings resident in fp8, split per kk for early start
        de_sb = res.tile([P, KK, 2, E], f8)
        for kk in range(KK):
            nc.sync.dma_start(de_sb[:, kk, :, :], de_d[:, kk, :, :])

        hs8_sb = res.tile([P, KK, 2, NT], f8)
        hsTb_sb = res.tile([P, HK, NT], bf16)

        y1T = res.tile([P, ICH, NT], bf16)         # silu(up-proj), I on partitions
        sT8 = res.tile([P, EC2, 2, NT], f8)        # S.T in DR-fp8 pair layout

        p8bf = res.tile([P, TCH, HEADS, 8], bf16)  # softmax probs (all heads)
        idx16 = res.tile([P, TCH, 4, HEADS, 8], i16)  # per-quarter scatter idx

        hsTb_r = hsT_b[:].rearrange("(o p) n -> p o n", p=P)
        for _rep in range(repeat):
            for kk in range(KK):
                nc.sync.dma_start(hs8_sb[:, kk, :, :], hs8_d[:, kk, :, :])
            nc.sync.dma_start(hsTb_sb[:, :HK // 2, :], hsTb_r[:, :HK // 2, :])
            nc.sync.dma_start(hsTb_sb[:, HK // 2:, :], hsTb_r[:, HK // 2:, :])

            # ---------- phase B: sim + all-expert logits, DoubleRow fp8.
            # kk-outer passes reuse the stationary hidden-state tile across
            # 3-4 matmuls per LDWEIGHTS; pass widths (4,3,3) keep at most 7
            # PSUM banks in flight (9 live banks on the 8-buf pool corrupts).
            for c in range(TCH):
                lg = lgpool.tile([P, E], bf16, tag="lg", name=f"lg{c}")
                hs_c = hs8_sb[:, :, :, c * P:(c + 1) * P]
                sim = rpool.tile([P, 512], f32, tag="sim")
                for ecs in ((-1, 0, 1, 2), (3, 4, 5), (6, 7)):
                    pss = [psum.tile([P, 512], f32, tag="ps", name=f"ps{c}_{ec}")
                           for ec in ecs]
                    for kk in range(KK):
                        for ps, ec in zip(pss, ecs):
                            mov = (wk_sb[:, kk, :, :] if ec < 0 else
                                   de_sb[:, kk, :, ec * 512:(ec + 1) * 512])
                            nc.tensor.matmul(ps[:], hs_c[:, kk, :, :], mov,
                                             start=(kk == 0), stop=(kk == KK - 1),
                                             perf_mode=DR)
                    for ps, ec in zip(pss, ecs):
                        if ec < 0:
                            # scores stay 64x-scaled; Exp descales later
                            nc.vector.tensor_copy(sim[:], ps[:])
                        elif ec % 2 == 0:
                            nc.scalar.activation(lg[:, ec * 512:(ec + 1) * 512],
                                                 ps[:], AF.Copy, scale=1.0 / FPS)
                        else:
                            # odd ec drains on DVE so bank recycling isn't
                            # serialized behind a single engine
                            nc.vector.tensor_scalar(
                                lg[:, ec * 512:(ec + 1) * 512], ps[:],
                                1.0 / FPS, None, op0=OP.mult)

                # ---- routing on the (64x-scaled) sim scores ----
                e8i = rpool.tile([P, HEADS, 8], i32, tag="e8i")
                for h in range(HEADS):
                    simx = sim[:, h * NK:(h + 1) * NK]
                    simy = sim[:, 256 + h * NK:256 + (h + 1) * NK]
                    sx = rpool.tile([P, 8], f32, tag="sx")
                    sy = rpool.tile([P, 8], f32, tag="sy")
                    ix = rpool.tile([P, 8], u32, tag="ix")
                    iy = rpool.tile([P, 8], u32, tag="iy")
                    nc.vector.max(sx[:], simx)
                    nc.vector.max_index(ix[:], sx[:], simx)
                    nc.vector.max(sy[:], simy)
                    nc.vector.max_index(iy[:], sy[:], simy)

                    cc = rpool.tile([P, 8, 8], f32, tag="cc")
                    nc.vector.tensor_tensor(cc[:], sx[:, :, None].to_broadcast([P, 8, 8]),
                                            sy[:, None, :].to_broadcast([P, 8, 8]), OP.add)
                    cflat = cc[:].rearrange("p a b -> p (a b)")
                    s8 = rpool.tile([P, 8], f32, tag="s8")
                    pk = rpool.tile([P, 8], u32, tag="pk")
                    nc.vector.max(s8[:], cflat)
                    nc.vector.max_index(pk[:], s8[:], cflat)

                    # softmax over the 8 selected scores (descale inside Exp)
                    d8 = rpool.tile([P, 8], f32, tag="d8")
                    nc.vector.tensor_scalar(d8[:], s8[:], s8[:, 0:1], None, op0=OP.subtract)
                    ex8 = rpool.tile([P, 8], f32, tag="ex8")
                    nc.scalar.activation(ex8[:], d8[:], AF.Exp, scale=1.0 / FPS)
                    z = rpool.tile([P, 1], f32, tag="z")
                    nc.vector.tensor_reduce(z[:], ex8[:], axis=AX.X, op=OP.add)
                    rz = rpool.tile([P, 1], f32, tag="rz")
                    nc.vector.reciprocal(rz[:], z[:])
                    nc.vector.tensor_scalar(p8bf[:, c, h, :], ex8[:], rz[:, 0:1], None,
                                            op0=OP.mult)

                    # expert ids: e8 = ix[pk>>3]*64 + iy[pk&7]
                    pkhu = rpool.tile([P, 8], u32, tag="pkhu")
                    pklu = rpool.tile([P, 8], u32, tag="pklu")
                    nc.vector.tensor_scalar(pkhu[:], pk[:], 3, None, op0=OP.logical_shift_right)
                    nc.vector.tensor_scalar(pklu[:], pk[:], 7, None, op0=OP.bitwise_and)
                    pkh = rpool.tile([P, 8], i32, tag="pkh")
                    pkl = rpool.tile([P, 8], i32, tag="pkl")
                    nc.vector.tensor_copy(pkh[:], pkhu[:])
                    nc.vector.tensor_copy(pkl[:], pklu[:])
                    ixi = rpool.tile([P, 8], i32, tag="ixi")
                    iyi = rpool.tile([P, 8], i32, tag="iyi")
                    nc.vector.tensor_copy(ixi[:], ix[:])
                    nc.vector.tensor_copy(iyi[:], iy[:])

                    ohx = rpool.tile([P, 8, 8], i32, tag="ohx")
                    ohy = rpool.tile([P, 8, 8], i32, tag="ohy")
                    nc.vector.tensor_tensor(ohx[:], pkh[:, :, None].to_broadcast([P, 8, 8]),
                                            iota8[:, None, :].to_broadcast([P, 8, 8]), OP.is_equal)
                    nc.vector.tensor_tensor(ohy[:], pkl[:, :, None].to_broadcast([P, 8, 8]),
                                            iota8[:, None, :].to_broadcast([P, 8, 8]), OP.is_equal)
                    mx = rpool.tile([P, 8, 8], i32, tag="mx")
                    my = rpool.tile([P, 8, 8], i32, tag="my")
                    nc.vector.tensor_tensor(mx[:], ohx[:],
                                            ixi[:, None, :].to_broadcast([P, 8, 8]), OP.mult)
                    nc.vector.tensor_tensor(my[:], ohy[:],
                                            iyi[:, None, :].to_broadcast([P, 8, 8]), OP.mult)
                    ixs = rpool.tile([P, 8], i32, tag="ixs")
                    iys = rpool.tile([P, 8], i32, tag="iys")
                    with nc.allow_low_precision(reason="int32 onehot-select, exact"):
                        nc.vector.tensor_reduce(ixs[:], mx[:], axis=AX.X, op=OP.add)
                        nc.vector.tensor_reduce(iys[:], my[:], axis=AX.X, op=OP.add)
                    nc.vector.scalar_tensor_tensor(e8i[:, h, :], ixs[:], NK, iys[:],
                                                   op0=OP.mult, op1=OP.add)

                # quarter-local scatter indices, wrong-quarter -> negative int16
                for q in range(4):
                    t1 = rpool.tile([P, HEADS, 8], i32, tag="t1")
                    nc.vector.tensor_scalar(t1[:], e8i[:], QE * q + QE, None,
                                            op0=OP.subtract)
                    t2 = rpool.tile([P, HEADS, 8], i32, tag="t2")
                    nc.vector.tensor_scalar(t2[:], t1[:], 4095, None,
                                            op0=OP.bitwise_and)
                    nc.vector.tensor_scalar(idx16[:, c, q, :, :], t2[:], 3072, None,
                                            op0=OP.subtract)

                # ---- phase A5: scatter all 4 heads at once, mult, transpose,
                #      silu straight into the fp8 S.T layout ----
                pq = scpool.tile([P, E], bf16, tag="pq")
                for q in range(4):
                    nc.gpsimd.local_scatter(
                        pq[:, q * QE:(q + 1) * QE], p8bf[:, c, :, :],
                        idx16[:, c, q, :, :], channels=P, num_elems=QE,
                        num_idxs=HEADS * 8)
                tt = scpool.tile([P, E], bf16, tag="tt")
                nc.vector.tensor_tensor(tt[:], lg[:], pq[:], OP.mult)
                stage = stpool.tile([P, ICH, P], bf16, tag="stage")
                # alternate dispatch queues so descriptor generation for the
                # transpose doesn't serialize behind one sequencer
                eng = nc.scalar if c % 2 == 0 else nc.sync
                eng.dma_start_transpose(stage[:], tt[:])
                nc.scalar.activation(
                    sT8[:, :, :, c * P:(c + 1) * P],
                    stage[:].rearrange("p (a b) m -> p a b m", b=2), AF.Silu)

            # ---------- phase C: dense up-proj + silu ----------
            for ic4 in range(ICH // 2):
                wup_t = streams.tile([P, HK, 2 * P], bf16, tag="wup_t")
                nc.sync.dma_start(wup_t[:], wupT_r[:, :, ic4 * 2 * P:(ic4 + 1) * 2 * P])
                for j in range(2):
                    ic = ic4 * 2 + j
                    ps = psum.tile([P, 512], f32, tag="ps")
                    for kk in range(HK):
                        nc.tensor.matmul(ps[:], wup_t[:, kk, j * P:(j + 1) * P],
                                         hsTb_sb[:, kk, :],
                                         start=(kk == 0), stop=(kk == HK - 1))
                    nc.scalar.activation(y1T[:, ic, :], ps[:], AF.Silu)

            # ---------- phase D: down-proj (bf16) + expert combine (DR fp8),
            #            fused in the same PSUM banks, [tok, d] out ----------
            # Interleave bf16 down-MM pairs with DR fp8 combine-MM pairs so
            # every fp8 LDWEIGHTS (256-col, no FWL, ~183ns) hides under the
            # neighboring bf16 matmuls via the PE reorder window. sT8 is
            # complete well before phase D starts (A5 ends during phase C).
            ps_d = [psum.tile([P, 512], f32, tag="ps", name=f"ps_d{c}_{dh}")
                    for c in range(TCH) for dh in range(2)]

            def down_mms(ic):
                wd_t = wstream.tile([P, H], bf16, tag="wd_t", name=f"wd_t{ic}")
                nc.sync.dma_start(wd_t[:], wdownT_r[:, ic, :])
                for c in range(TCH):
                    for dh in range(2):
                        nc.tensor.matmul(ps_d[c * 2 + dh][:],
                                         y1T[:, ic, c * P:(c + 1) * P],
                                         wd_t[:, dh * 512:(dh + 1) * 512],
                                         start=(ic == 0), stop=(ic == ICH - 1))

            def comb_mms(ic2):
                ue_t = wstream.tile([P, 2, H], f8, tag="ue_t", name=f"ue_t{ic2}")
                # ACT's DMA queue is idle during phase D; wd stays on SP so
                # the two weight streams generate descriptors in parallel
                nc.scalar.dma_start(ue_t[:], ue_d[:, ic2, :, :])
                for c in range(TCH):
                    for dh in range(2):
                        nc.tensor.matmul(ps_d[c * 2 + dh][:],
                                         sT8[:, ic2, :, c * P:(c + 1) * P],
                                         ue_t[:, :, dh * 512:(dh + 1) * 512],
                                         start=False, stop=False,
                                         perf_mode=DR)

            for k in range(EC2):
                down_mms(2 * k)
                comb_mms(k)
                down_mms(2 * k + 1)
            for c in range(TCH):
                for dh in range(2):
                    ot = outp.tile([P, 512], f32, tag="ot")
                    if dh == 0:
                        nc.vector.tensor_copy(ot[:], ps_d[c * 2 + dh][:])
                    else:
                        nc.scalar.activation(ot[:], ps_d[c * 2 + dh][:], AF.Copy)
                    oeng = nc.gpsimd if dh == 0 else nc.sync
                    oeng.dma_start(
                        out_d[c * P:(c + 1) * P, dh * 512:(dh + 1) * 512], ot[:])

    nc.compile()
    return nc


def _host_prep(hidden_states, W_up, W_down, W_q, keys, down_embed, up_embed):
    bf = ml_dtypes.bfloat16
    f8 = ml_dtypes.float8_e4m3
    hs = np.asarray(hidden_states, dtype=np.float32).reshape(B * T, H)
    W_up = np.asarray(W_up, dtype=np.float32)
    W_down = np.asarray(W_down, dtype=np.float32)
    W_q = np.asarray(W_q, dtype=np.float32)
    keys = np.asarray(keys, dtype=np.float32)
    down_embed = np.asarray(down_embed, dtype=np.float32)
    up_embed = np.asarray(up_embed, dtype=np.float32)

    # compose product-key similarity: WK[(p2,h,k), d] = sum_r Wq[(p2,h,r), d]*keys[h,k,p2,r]
    Wq3 = W_q.reshape(2, HEADS, NK, H).astype(np.float64)
    WK = np.einsum("phrd,hkpr->phkd", Wq3, keys.astype(np.float64))
    WK_T = np.ascontiguousarray(WK.reshape(512, H).T.astype(np.float32))  # [H, 512]

    def dr_pack(m):  # [H, N] -> [128, KK, 2, N] with h = kk*256 + i*128 + p
        return np.ascontiguousarray(
            m.reshape(KK, 2, P, m.shape[1]).transpose(2, 0, 1, 3))

    shared = {
        "wk8": dr_pack(WK_T * FPS).astype(f8),
        "de8": dr_pack(np.ascontiguousarray(down_embed.T) * FPS).astype(f8),
        "wupT": np.ascontiguousarray(W_up.T).astype(bf),            # [H, I]
        "wdownT": np.ascontiguousarray(W_down.T).astype(bf),        # [I, H]
        # [128, EC2, 2, H] with e = ic2*256 + i*128 + p, raw scale
        "ue8": np.ascontiguousarray(
            up_embed.reshape(EC2, 2, P, H).transpose(2, 0, 1, 3)).astype(f8),
    }
    in_maps = []
    for i in range(NCORES):
        shard = hs[i * NT:(i + 1) * NT]                              # [NT, H]
        hsT = np.ascontiguousarray(shard.T)                          # [H, NT]
        m = dict(shared)
        m["hs8"] = dr_pack(hsT).astype(f8)
        m["hsT_b"] = hsT.astype(bf)
        in_maps.append(m)
    return in_maps


def kernel(hidden_states, W_up, W_down, W_q, keys, down_embed, up_embed,
           trace=False):
    from concourse.bass_utils import run_bass_kernel_spmd

    if "nc" not in _CACHE:
        _CACHE["nc"] = _build_program()
    nc = _CACHE["nc"]

    in_maps = _host_prep(hidden_states, W_up, W_down, W_q, keys,
                         down_embed, up_embed)
    res = run_bass_kernel_spmd(nc, in_maps, list(range(NCORES)), trace=trace)
    out = np.empty((B * T, H), np.float32)
    for i, r in enumerate(res.results):
        out[i * NT:(i + 1) * NT] = r["out"]
    if trace:
        kernel.last_results = res
    return out.reshape(B, T, H)
